# revision 7
# baseline (speedup 1.0000x reference)
"""Trainium2 Bass kernel for SimCLR-style contrastive (NT-Xent) loss.

Reference computation:
    z = concat(emb_i, emb_j)            # [8192, 256]
    z = z / ||z||_row
    sim = (z @ z.T) / 0.5               # [8192, 8192]
    sim[i, i] = -inf
    loss = mean_i( logsumexp_j(sim[i, :]) - sim[i, label_i] )
    label_i = (i + 4096) % 8192

Distribution: symmetric cyclic-band sharding. Core c owns global rows
[1024c, 1024c+1024) (host np.roll makes the SPMD program uniform: its rows
are always local rows 0..1023). Each core computes exp(sim) only for the
cyclic band of tile-blocks (t, t+k), k = 0..32, t = local row tile 0..7 —
half the matrix globally. Per-row softmax denominators are assembled from
  - row sums of all band blocks (ACT accumulator), and
  - column sums of blocks k = 1..31 (TensorE ones-matmuls), which supply
    the mirrored lower-triangle contributions of OTHER cores' rows.
Element-exact coverage: for a pair {i, j} at tile distance k vs 64-k, the
row-sum band [0,32] and col-sum band [1,31] contribute exactly once each
(k' = 64-k), including the d=4096 positive pairs (k=32 from both sides).

Precision: matmuls run in fp8e4 (DoubleRow, 0.5 cyc/row). lhsT row tiles
are plane-major (PE transpose); the big rhs is built by one DMA xbar
transpose of fp8 PAIRS viewed as uint16, giving byte-interleaved K pairs
(k = 2p+j), which the moving side accepts. exp outputs are fp8
pair-interleaved so column sums of two row-tiles run as one DoubleRow
ones-matmul (0.25 cyc/element).

Host combines per-core partial row/col sums in float64 and applies the
final ln (24K flops vs 17 GFLOP on device).
"""

import os
import sys
from contextlib import ExitStack

import numpy as np

for _p in ("/opt/trn_rl_repo",):
    if os.path.isdir(_p) and _p not in sys.path:
        sys.path.insert(0, _p)

import concourse.bacc as bacc
import concourse.tile as tile
from concourse import mybir
from concourse.bass_utils import run_bass_kernel_spmd
from concourse.masks import make_identity

F32 = mybir.dt.float32
FP8 = mybir.dt.float8e4
U16 = mybir.dt.uint16
AF = mybir.ActivationFunctionType
ALU = mybir.AluOpType
DR = mybir.MatmulPerfMode.DoubleRow

N, D = 8192, 256          # 2B rows, feature dim
NCORES = 8
ROWS = N // NCORES        # 1024 rows owned per core
RT = ROWS // 128          # 8 local row tiles
BANDK = 33                # tile-block band k = 0..32
NCT = RT - 1 + BANDK      # 40 column tiles each core loads (0..39)
NLC = NCT * 128           # 5120 local columns
BANDW = 4224              # per-row band width in columns (33 tiles)
CHUNK = 1536              # psum gram chunk (3 banks)

_ACT_SET = "natural_log_exp_and_others"


def _patch_act_tables():
    """Restrict the ACT table-set chooser to the one set containing every
    function this kernel uses (Exp, Ln), avoiding ACT_TABLE_LOAD churn."""
    if getattr(bacc, "_act_tables_patched", False):
        return
    orig = bacc.get_activation_tables

    def restricted(arch):
        full = dict(orig(arch))
        return {
            name: (fns if name == _ACT_SET else set())
            for name, fns in full.items()
        }

    bacc.get_activation_tables = restricted
    bacc._act_tables_patched = True


def _segs(lo, hi, step, align0=0):
    """Split [lo, hi) at multiples of `step` relative to align0."""
    out = []
    x = lo
    while x < hi:
        nx = min(hi, ((x - align0) // step + 1) * step + align0)
        out.append((x, nx))
        x = nx
    return out


def _build_kernel(ctx, tc, z, rows_out, cols_out):
    nc = tc.nc
    v = nc.vector
    s = nc.scalar
    te = nc.tensor
    gp = nc.gpsimd
    sy = nc.sync

    zr = z.rearrange("(t p) d -> p t d", p=128)  # [128, 40, 256] DRAM view

    pers = ctx.enter_context(tc.tile_pool(name="pers", bufs=1))
    stg = ctx.enter_context(tc.tile_pool(name="stg", bufs=5))
    epool = ctx.enter_context(tc.tile_pool(name="epool", bufs=6))
    csp = ctx.enter_context(tc.tile_pool(name="csp", bufs=2))

    zn8 = pers.tile([128, NCT, D], FP8)          # normalized fp8, row-major
    znT2 = pers.tile([128, NCT, 128], U16)       # pair-transposed (k = 2p+j)
    lhsT = pers.tile([128, RT, 2, 128], FP8)     # plane-major row tiles
    ss = pers.tile([128, NCT], F32)
    lss = pers.tile([128, NCT], F32)
    rinv = pers.tile([128, NCT], F32)
    sqjunk = pers.tile([128, D], F32)
    dotjunk = pers.tile([128, D], FP8)
    sparts = pers.tile([128, RT * 3], F32)       # ACT accum slots (row, chunk)
    selfs = pers.tile([128, RT], F32)            # 2*selfdot per own row
    finals = pers.tile([128, 2 * RT], F32)       # [rowsum_adj | numer]
    rowsum = pers.tile([128, RT], F32)
    ediag = pers.tile([128, RT], F32)
    negtwo = pers.tile([128, 1], F32)
    ones8 = pers.tile([128, 2, 16], FP8)         # DR colsum weights (step 16)
    ones1 = pers.tile([128, 16], FP8)            # solo colsum weights
    ident = pers.tile([128, 128], FP8)

    v.memset(negtwo[:], -2.0)
    v.memset(ones8[:], 1.0)
    v.memset(ones1[:], 1.0)
    make_identity(nc, ident[:])

    # ---- Phase 1: load + sumsq + rinv + normalize/cast + pair-transpose ----
    sts = []
    for g in range(5):
        st = stg.tile([128, 8, D], F32, tag="st", name="st")
        for q in range(4):
            sy.dma_start(
                st[:, q * 2:(q + 1) * 2, :],
                zr[:, g * 8 + q * 2:g * 8 + (q + 1) * 2, :],
            )
        sts.append(st)

    def emit_rinv(t0, t1):
        sl = slice(t0, t1)
        s.activation(lss[:, sl], ss[:, sl], AF.Ln)
        s.activation(rinv[:, sl], lss[:, sl], AF.Exp, scale=-0.5)

    for g in range(5):
        st = sts[g]
        for i in range(8):
            t = g * 8 + i
            v.scalar_tensor_tensor(
                out=sqjunk[:], in0=st[:, i, :], scalar=1.0, in1=st[:, i, :],
                op0=ALU.mult, op1=ALU.mult, accum_out=ss[:, t:t + 1],
            )
        if g % 2 == 1 or g == 4:
            emit_rinv((g // 2) * 16, g * 8 + 8)
        if g % 2 == 1 or g == 4:
            for gg in (g - 1, g) if g % 2 == 1 else (g,):
                for i in range(8):
                    t = gg * 8 + i
                    gp.tensor_scalar_mul(
                        zn8[:, t, :], sts[gg][:, i, :], rinv[:, t:t + 1]
                    )
                zu = zn8[:, gg * 8:(gg + 1) * 8, :].bitcast(U16).rearrange(
                    "p t d -> p (t d)"
                )
                sy.dma_start_transpose(znT2[:, gg * 8:(gg + 1) * 8, :], zu)

    # byte-interleaved fp8 view of the transposed z: [128, 2, 5120]
    rhsv = znT2.bitcast(FP8).rearrange("p t (c j) -> p j (t c)", j=2)

    # ---- Phase 1b: self/pair dots + plane-major lhsT ----
    for r in range(RT):
        v.scalar_tensor_tensor(
            out=dotjunk[:], in0=zn8[:, r, :], scalar=2.0, in1=zn8[:, r, :],
            op0=ALU.mult, op1=ALU.mult, accum_out=selfs[:, r:r + 1],
        )
        v.scalar_tensor_tensor(
            out=dotjunk[:], in0=zn8[:, r, :], scalar=2.0, in1=zn8[:, r + 32, :],
            op0=ALU.mult, op1=ALU.mult,
            accum_out=finals[:, RT + r:RT + r + 1],
        )

    with tc.tile_pool(name="tpp", bufs=1, space="PSUM") as tpp:
        tps = tpp.tile([128, 512], FP8)
        t3 = tps.rearrange("p (h c j) -> p h c j", h=2, j=2)
        for r in range(RT):
            for j in range(2):
                zsl = zn8[:, r, :].rearrange("p (c j) -> p j c", j=2)[:, j, :]
                te.transpose(t3[:, j, :, 0], zsl, ident[:])
            v.tensor_copy(
                lhsT[:, r, :, :],
                t3[:, :, :, 0],
            )

    # ---- Phase 2: band gram + exp + colsums, one row-pair at a time ----
    pg = ctx.enter_context(tc.tile_pool(name="pg", bufs=2, space="PSUM"))
    pc = ctx.enter_context(tc.tile_pool(name="pc", bufs=1, space="PSUM"))

    for pi in range(RT // 2):
        a = 2 * pi
        base = a * 128                       # local col of rel 0
        # rel coverage: row a [0, 4224), row b [128, 4352)
        cov = {a: (0, BANDW), a + 1: (128, BANDW + 128)}
        echunks = []
        for k in range(3):
            ck0, ck1 = k * CHUNK, min((k + 1) * CHUNK, BANDW + 128)
            e8 = epool.tile([128, 2, CHUNK], FP8, tag="e8", name="e8")
            echunks.append((ck0, ck1, e8))

        for r in (a, a + 1):
            j = r - a
            for k in range(3):
                ck0, ck1, e8 = echunks[k]
                lo = max(cov[r][0], ck0)
                hi = min(cov[r][1], ck1)
                pgt = pg.tile([128, CHUNK], F32, tag="pg", name="pg")
                for s0, s1 in _segs(lo - ck0, hi - ck0, 512):
                    te.matmul(
                        pgt[:, s0:s1], lhsT[:, r, :, :],
                        rhsv[:, :, base + ck0 + s0:base + ck0 + s1],
                        start=True, stop=True, perf_mode=DR,
                    )
                s.activation(
                    e8[:, j, lo - ck0:hi - ck0], pgt[:, lo - ck0:hi - ck0],
                    AF.Exp, bias=negtwo[:, 0:1], scale=2.0,
                    accum_out=sparts[:, r * 3 + k:r * 3 + k + 1],
                )

        # colsums: slots s cover rel [128+512s, 640+512s), s = 0..7.
        # pair region rel [256, 4096); solo-a [128, 256); solo-b [4096, 4224).
        # PE col-tiling quadrant 3 (partition 96) is unusable, so 6 slots
        # fit the two colsum banks; slots 6-7 reuse bank 6 after a flush.
        cpt = pc.tile([128, 1024], F32, tag="cp", name="cp")
        for sl in range(8):
            if sl == 6:
                # flush slots 0-5, then reuse the tile for the tail slots
                csb = csp.tile([128, 1024], F32, tag="csb", name="csb")
                v.tensor_copy(csb[:], cpt[:])
                sy.dma_start(cols_out[pi, 0:3, :], csb[0:65:32, :])
                cpt = pc.tile([128, 1024], F32, tag="cp", name="cp")
            r0, r1 = 128 + 512 * sl, 640 + 512 * sl
            if sl < 6:
                po, co = 32 * (sl % 3), 512 * (sl // 3)
            else:
                po, co = 32 * (sl - 6), 0
            x = r0
            while x < r1:
                k = min(x // CHUNK, 2)
                ck0, ck1, e8 = echunks[k]
                px = min(r1, ck1)
                # split piece [x, px) into pair/solo parts
                parts = []
                if x < 256:
                    parts.append((x, min(px, 256), "a"))
                if max(x, 256) < min(px, 4096):
                    parts.append((max(x, 256), min(px, 4096), "p"))
                if max(x, 4096) < px:
                    parts.append((max(x, 4096), px, "b"))
                for p0, p1, kind in parts:
                    o0 = co + (p0 - r0)
                    out = cpt[po:po + 1, o0:o0 + p1 - p0]
                    if kind == "p" and po == 0:
                        # DoubleRow pair colsum: dst partition must be 0
                        te.matmul(
                            out, ones8[:, :, 0:1],
                            e8.rearrange("p j c -> p j c")[
                                :, :, p0 - ck0:p1 - ck0
                            ],
                            start=True, stop=True, perf_mode=DR,
                            tile_position=(0, po),
                        )
                    elif kind == "p":
                        # two accumulating solo colsums (one per row plane)
                        for jj in range(2):
                            te.matmul(
                                out, ones1[:, 0:1],
                                e8[:, jj, p0 - ck0:p1 - ck0],
                                start=(jj == 0), stop=(jj == 1),
                                tile_position=(0, po),
                            )
                    else:
                        jj = 0 if kind == "a" else 1
                        te.matmul(
                            out, ones1[:, 0:1],
                            e8[:, jj, p0 - ck0:p1 - ck0],
                            start=True, stop=True,
                            tile_position=(0, po),
                        )
                x = px
        csb = csp.tile([128, 1024], F32, tag="csb", name="csb")
        v.tensor_copy(csb[:, 0:512], cpt[:, 0:512])
        sy.dma_start(
            cols_out[pi, 3, :].rearrange("(a b) -> a b", a=2),
            csb[0:33:32, 0:512],
        )

    # ---- Phase 3: finals ----
    v.tensor_reduce(
        rowsum[:], sparts.rearrange("p (r k) -> p r k", k=3),
        axis=mybir.AxisListType.X, op=ALU.add,
    )
    s.activation(ediag[:], selfs[:], AF.Exp, bias=negtwo[:, 0:1])
    v.tensor_sub(finals[:, 0:RT], rowsum[:], ediag[:])
    sy.dma_start(rows_out[:], finals[:])


_CACHE = {}


def get_nc():
    if "nc" not in _CACHE:
        _patch_act_tables()
        nc = bacc.Bacc(
            "TRN2", target_bir_lowering=False, debug=False, num_devices=NCORES
        )
        z = nc.dram_tensor("z", [NLC, D], F32, kind="ExternalInput").ap()
        rows_out = nc.dram_tensor(
            "rows_out", [128, 2 * RT], F32, kind="ExternalOutput"
        ).ap()
        cols_out = nc.dram_tensor(
            "cols_out", [RT // 2, 4, 1024], F32, kind="ExternalOutput"
        ).ap()
        with tile.TileContext(nc) as tc:
            with ExitStack() as ctx:
                _build_kernel(ctx, tc, z, rows_out, cols_out)
        nc.compile()
        _CACHE["nc"] = nc
    return _CACHE["nc"]


def make_in_maps(embeddings_i, embeddings_j):
    ei = np.ascontiguousarray(np.asarray(embeddings_i), dtype=np.float32)
    ej = np.ascontiguousarray(np.asarray(embeddings_j), dtype=np.float32)
    z = np.concatenate([ei, ej], axis=0)
    return [
        {"z": np.ascontiguousarray(np.roll(z, -ROWS * c, axis=0)[:NLC])}
        for c in range(NCORES)
    ]


def reduce_results(results):
    S = np.zeros(N, dtype=np.float64)
    numer = np.zeros(N, dtype=np.float64)
    cols = np.arange(512)
    for c, r in enumerate(results):
        o = ROWS * c
        rows_out = r["rows_out"].astype(np.float64)   # [128, 16]
        idx = (o + np.arange(ROWS)) % N               # local row t*128+p
        rs = rows_out[:, 0:RT].T.reshape(ROWS)        # [t, p] -> t*128+p
        nu = rows_out[:, RT:2 * RT].T.reshape(ROWS)
        S[idx] += rs
        numer[idx] = nu
        cols_out = r["cols_out"].astype(np.float64)   # [4, 4, 1024]
        for pi in range(RT // 2):
            a = 2 * pi
            for sl in range(8):
                L = a * 128 + 128 + 512 * sl
                if sl < 6:
                    vals = cols_out[pi, sl % 3,
                                    512 * (sl // 3):512 * (sl // 3) + 512]
                else:
                    vals = cols_out[pi, 3, 512 * (sl - 6):512 * (sl - 6) + 512]
                S[(o + L + cols) % N] += vals
    loss = np.mean(np.log(S) + 2.0 - numer)
    return np.float32(loss)


def run(embeddings_i, embeddings_j, **spmd_kwargs):
    nc = get_nc()
    in_maps = make_in_maps(embeddings_i, embeddings_j)
    res = run_bass_kernel_spmd(nc, in_maps, list(range(NCORES)), **spmd_kwargs)
    return reduce_results(res.results), res


def kernel(embeddings_i, embeddings_j):
    loss, _ = run(embeddings_i, embeddings_j)
    return loss
